# revision 30
# baseline (speedup 1.0000x reference)
"""BiLSTM (2-layer, masked/ragged) Trainium2 kernel.

Sharding: 8 cores = 2 directions x 4 batch shards (16 each). Backward cores
receive time-reversed inputs from the host, so the device program is
direction-agnostic SPMD. Layer-0 outputs are exchanged between fwd/bwd
partner cores with pairwise AllGathers; each core computes layer-1 input
projections from its own half plus a time-reversed read of the partner half.

Scan step structure (per timestep, PSUM bank-parallel):
  - xw (precomputed input projection + bias) is injected into the four
    per-gate PSUM accumulators with identity matmuls (no DVE add).
  - 64 Whh matmuls (free dim 16) accumulate over it, ordered g,i,f,o so the
    sigmoid/tanh chain for c_t overlaps the tail of the PE stream.
  - Activations split per gate; h_{t+1}'s masked state is produced as
    (sigmoid(o)*mask) * tanh(c) so only one DVE op trails the last tanh.
Whh and the identity are fp8e4m3 (scaled x64, un-scaled via the free ACT
`scale` operand): FWL makes fp8 LDWEIGHTS 2x faster than bf16, and the
64-tile weight reload per step is the PE-side bound of the recurrence
(free-dim-16 matmuls are LDW-bound, not FLOP-bound). Wih/x/h stay bf16
(mixed fp8-stationary x bf16-moving matmuls are exact on TRN2); cell state
and the elementwise chain are fp32. Proj psum->SBUF activations are split
in halves so they can't head-of-line-block the scan's chain activations on
the ACT FIFO. DMAs batched 8 steps.
"""

import numpy as np
import ml_dtypes

import concourse.bass as bass
import concourse.bacc as bacc
import concourse.mybir as mybir
import concourse.tile as tile
from concourse import bass_utils

bf16 = ml_dtypes.bfloat16
f8 = ml_dtypes.float8_e4m3
f32 = mybir.dt.float32
bf = mybir.dt.bfloat16
fp8 = mybir.dt.float8e4
WSCALE = 64.0  # Whh stored as fp8e4m3 * 64; un-scaled in the gate activations

T, B, D, H = 512, 64, 512, 512
NCORES = 8
BS = B // 4          # 16 batch per core
G = 4 * H            # 2048 gates
GT = G // 128        # 16 gate tiles
KH = H // 128        # 4 k-chunks
HB = KH * BS         # 64 state cols
SC = 8               # scan steps per DMA batch
TC = 32              # proj timesteps per chunk
NCH = T // TC

SIG = mybir.ActivationFunctionType.Sigmoid
TANH = mybir.ActivationFunctionType.Tanh
IDENT = mybir.ActivationFunctionType.Identity

_compiled = {}


def _build(t_steps=T, dbg=False, nocc=False, parts="all", interleave=True, prefetch=True):
    nc = bacc.Bacc("TRN2", target_bir_lowering=False, debug=False,
                   num_devices=NCORES)
    nchunks = t_steps // TC

    xT = nc.dram_tensor("xT", (D, t_steps, BS), bf, kind="ExternalInput")
    maskd = nc.dram_tensor("maskd", (t_steps + 1, 128, HB), bf, kind="ExternalInput")
    whh0T = nc.dram_tensor("whh0T", (KH, 128, G), fp8, kind="ExternalInput")
    wih0T = nc.dram_tensor("wih0T", (KH, 128, G), bf, kind="ExternalInput")
    whh1T = nc.dram_tensor("whh1T", (KH, 128, G), fp8, kind="ExternalInput")
    wih1oT = nc.dram_tensor("wih1oT", (KH, 128, G), bf, kind="ExternalInput")
    wih1pT = nc.dram_tensor("wih1pT", (KH, 128, G), bf, kind="ExternalInput")
    identT = nc.dram_tensor("identT", (128, 128), fp8, kind="ExternalInput")
    b0c = nc.dram_tensor("b0c", (GT, 128), f32, kind="ExternalInput")
    b1c = nc.dram_tensor("b1c", (GT, 128), f32, kind="ExternalInput")
    y1 = nc.dram_tensor("y1", (t_steps, 128, HB), bf, kind="ExternalOutput")
    if dbg:
        y0o = nc.dram_tensor("y0o", (t_steps, 128, HB), bf, kind="ExternalOutput")
    with tile.TileContext(nc) as tc:
        with (
            tc.tile_pool(name="wpool", bufs=1) as wpool,
            tc.tile_pool(name="xpool", bufs=3) as xpool,
            tc.tile_pool(name="xcpool", bufs=4) as xcpool,
            tc.tile_pool(name="gpool", bufs=3) as gpool,
            tc.tile_pool(name="spool", bufs=6) as spool,
            tc.tile_pool(name="opool", bufs=2) as opool,
            tc.tile_pool(name="mpool", bufs=2) as mpool,
            tc.tile_pool(name="state", bufs=1) as state,
            tc.tile_pool(name="psA", bufs=3, space="PSUM") as psA,
            tc.tile_pool(name="psS", bufs=1, space="PSUM") as psS,
            tc.tile_pool(name="dram", bufs=1, space="DRAM") as dram,
        ):
            y0 = dram.tile([t_steps, 128, HB], bf)
            ag = dram.tile([2 * t_steps, 128, HB], bf)

            def load_w(name, src, dt=bf):
                t = wpool.tile([128, KH * G], dt, tag=name)
                for k in range(KH):
                    nc.sync.dma_start(t[:, k * G:(k + 1) * G], src.ap()[k])
                return t

            whh0_sb = load_w("whh0", whh0T, fp8)
            wih0_sb = load_w("wih0", wih0T)
            whh1_sb = load_w("whh1", whh1T, fp8)
            wih1o_sb = load_w("wih1o", wih1oT)
            wih1p_sb = load_w("wih1p", wih1pT)
            ident_sb = wpool.tile([128, 128], fp8, tag="ident")
            nc.sync.dma_start(ident_sb[:], identT.ap())
            bias_sb = wpool.tile([128, 2 * GT], f32, tag="bias")
            nc.sync.dma_start(bias_sb[:, 0:GT], b0c.ap().transpose([1, 0]))
            nc.sync.dma_start(bias_sb[:, GT:2 * GT], b1c.ap().transpose([1, 0]))

            # ---- input projections -> xwb dram ----
            # Emitted as a list of small "quanta" (thunks) so chunks beyond
            # the first two can be interleaved into the recurrent scan's PE
            # bubbles (the scan waits ~1us per step on the h-chain; proj
            # matmuls have no h dependency and fill that idle time).
            def proj_quanta(chmap, w_sbs, srcs, bias_col, chunks):
                nk = len(w_sbs) * KH
                quanta = []
                for ch in chunks:
                    t0 = ch * TC
                    state = {}

                    def dma_q(ch=ch, t0=t0, state=state):
                        rhs = xpool.tile([128, nk, TC, BS], bf, tag="projx",
                                         name="projx")
                        ji = 0
                        for w_sb, src in zip(w_sbs, srcs):
                            for k in range(KH):
                                nc.sync.dma_start(rhs[:, ji], src(k, t0))
                                ji += 1
                        state["rhs"] = rhs
                        # SBUF-resident destination chunk: Tile tracks SBUF
                        # deps reliably, making scan-interleaved production
                        # race-free (and skipping the xwb DRAM round trip).
                        state["xc"] = xcpool.tile([128, TC, GT * BS], bf,
                                                  tag="xchunk", name="xchunk")
                        chmap[ch] = state["xc"]
                    quanta.append(dma_q)

                    for g in range(GT):
                        halves = ([range(0, nk)] if nk <= 4 else
                                  [range(0, 4), range(4, nk)])
                        for hi, js in enumerate(halves):
                            def gate_q(t0=t0, g=g, js=js, hi=hi, nh=len(halves),
                                       state=state):
                                if hi == 0:
                                    state["ps"] = psA.tile(
                                        [128, TC * BS], f32, tag="psA",
                                        name="psA")
                                ps = state["ps"]
                                rhs = state["rhs"]
                                for ji in js:
                                    w_sb = w_sbs[ji // KH]
                                    k = ji % KH
                                    nc.tensor.matmul(
                                        ps[:],
                                        w_sb[:, k * G + g * 128: k * G + (g + 1) * 128],
                                        rhs[:, ji],
                                        start=(ji == 0),
                                        stop=(ji == nk - 1),
                                    )
                                if hi == nh - 1:
                                    # two half-ops: a 600ns ACT head-of-line-
                                    # blocks the scan chain's activations
                                    T2 = TC // 2
                                    for th in range(2):
                                        nc.scalar.activation(
                                            state["xc"][:, th * T2:(th + 1) * T2,
                                                        g * BS:(g + 1) * BS],
                                            ps[:, th * T2 * BS:(th + 1) * T2 * BS]
                                            .rearrange("p (t b) -> p t b", t=T2),
                                            IDENT,
                                            bias=bias_sb[:, bias_col + g:
                                                         bias_col + g + 1],
                                        )
                            quanta.append(gate_q)
                return quanta

            def emit_all(quanta):
                for q in quanta:
                    q()

            srcs0 = [lambda k, t0: xT.ap()[k * 128:(k + 1) * 128, t0:t0 + TC, :]]
            xc0, xc1 = {}, {}
            projA_rest = []
            if parts in ("all", "proj"):
                nup = min(2, nchunks) if (interleave and parts == "all") else nchunks
                emit_all(proj_quanta(xc0, [wih0_sb], srcs0, 0, range(nup)))
                projA_rest = proj_quanta(xc0, [wih0_sb], srcs0, 0,
                                         range(nup, nchunks))

            # ---- recurrent scan ----
            # natural gate-type order in xw columns / weight columns: i,f,g,o
            QI = {"i": 0, "f": 1, "g": 2, "o": 3}

            def scan(chmap, whh_sb, y_dst, extra=()):
                h0 = state.tile([128, HB], bf, tag="h0")
                cst = state.tile([128, HB], f32, tag="cst")
                nc.gpsimd.memset(h0[:], 0.0)
                nc.gpsimd.memset(cst[:], 0.0)
                hprev = h0[:]
                extra = list(extra)
                nex = len(extra)
                spread = max(1, t_steps - 2 * TC)

                def load_win(t0):
                    mk = mpool.tile([128, SC + 1, HB], bf, tag="mk", name="mk")
                    nc.sync.dma_start(
                        mk[:], maskd.ap()[t0:t0 + SC + 1].transpose([1, 0, 2]))
                    return mk

                nxt = load_win(0) if prefetch else None
                mk = ob = None
                for t in range(t_steps):
                    j = t % SC
                    if j == 0:
                        if prefetch:
                            mk = nxt
                            if t + SC < t_steps:
                                nxt = load_win(t + SC)
                        else:
                            mk = load_win(t)
                        ob = opool.tile([128, SC, HB], bf, tag="ob", name="ob")
                    xw = chmap[t // TC]
                    tr = t % TC
                    # c state mask (c *= m[t]); h mask was folded into hprev
                    if t > 0:
                        nc.vector.tensor_mul(cst[:], cst[:], mk[:, j])
                    ps = {q: psS.tile([128, 512], f32, tag=f"ps{q}",
                                      name=f"ps{q}") for q in "ifgo"}
                    for q in "ifgo":
                        nc.tensor.matmul(
                            ps[q][:, 0:HB], ident_sb[:],
                            xw[:, tr, QI[q] * HB:(QI[q] + 1) * HB],
                            start=True, stop=False, skip_group_check=True)
                    for q in "gifo":
                        qi = QI[q]
                        for gt in range(4):
                            for k in range(KH):
                                nc.tensor.matmul(
                                    ps[q][:, gt * BS:(gt + 1) * BS],
                                    whh_sb[:, k * G + qi * 512 + gt * 128:
                                           k * G + qi * 512 + (gt + 1) * 128],
                                    hprev[:, k * BS:(k + 1) * BS],
                                    start=False, stop=(k == KH - 1),
                                    skip_group_check=True)
                    inv = 1.0 / WSCALE
                    tg = spool.tile([128, HB], f32, tag="tg")
                    nc.scalar.activation(tg[:], ps["g"][:, 0:HB], TANH, scale=inv)
                    si = spool.tile([128, HB], f32, tag="si")
                    nc.scalar.activation(si[:], ps["i"][:, 0:HB], SIG, scale=inv)
                    ig = spool.tile([128, HB], f32, tag="ig")
                    nc.vector.tensor_mul(ig[:], si[:], tg[:])
                    sf = spool.tile([128, HB], f32, tag="sf")
                    nc.scalar.activation(sf[:], ps["f"][:, 0:HB], SIG, scale=inv)
                    fc = spool.tile([128, HB], f32, tag="fc")
                    nc.vector.tensor_mul(fc[:], sf[:], cst[:])
                    nc.vector.tensor_add(cst[:], fc[:], ig[:])
                    so = spool.tile([128, HB], f32, tag="so")
                    nc.scalar.activation(so[:], ps["o"][:, 0:HB], SIG, scale=inv)
                    tc2 = spool.tile([128, HB], f32, tag="tc2")
                    nc.scalar.activation(tc2[:], cst[:], TANH)
                    if t + 1 < t_steps:
                        # next-state path first: hm = (so*m[t+1]) * tanh(c)
                        som = spool.tile([128, HB], f32, tag="som")
                        nc.vector.tensor_mul(som[:], so[:], mk[:, j + 1])
                        hm = spool.tile([128, HB], bf, tag="hm")
                        nc.vector.tensor_mul(hm[:], som[:], tc2[:])
                        hprev = hm[:]
                    h2 = spool.tile([128, HB], f32, tag="h2")
                    nc.vector.tensor_mul(h2[:], so[:], tc2[:])
                    nc.vector.tensor_mul(ob[:, j], h2[:], mk[:, j])
                    if j == SC - 1:
                        nc.sync.dma_start(
                            y_dst[t - SC + 1:t + 1].transpose([1, 0, 2]), ob[:])
                    # fill this step's PE bubble with interleaved proj work
                    q0 = nex * t // spread
                    q1 = nex * (t + 1) // spread
                    for q in extra[q0:min(q1, nex)]:
                        q()

            if parts in ("all", "scans"):
                scan(xc0, whh0_sb, y0, extra=projA_rest)
            if dbg:
                nc.sync.dma_start(y0o.ap()[:], y0[:])

            # ---- exchange (pairwise fwd<->bwd) ----
            projD_rest = []
            if parts in ("all", "proj"):
                if nocc:
                    nc.sync.dma_start(ag[0:t_steps], y0[:])
                    nc.sync.dma_start(ag[t_steps:2 * t_steps], y0[:])
                    partner_row = nc.snap(t_steps)
                else:
                    nc.gpsimd.collective_compute(
                        "AllGather", mybir.AluOpType.bypass,
                        ins=[y0.opt()], outs=[ag.opt()],
                        replica_groups=[[0, 4], [1, 5], [2, 6], [3, 7]],
                    )
                    partner_row = nc.snap(
                        ((nc.partition_id() // 4 + 1) % 2) * t_steps)

                def par_src(k, t0):
                    # partner rows, time-reversed: own step tau needs partner
                    # row (T-1-tau); rows [T-TC-t0, T-t0) reversed.
                    return (ag[bass.ds(partner_row + (t_steps - TC - t0), TC)]
                            [::-1, :, k * BS:(k + 1) * BS].transpose([1, 0, 2]))

                srcs1 = [
                    lambda k, t0: y0[t0:t0 + TC, :, k * BS:(k + 1) * BS].transpose([1, 0, 2]),
                    par_src,
                ]
                nup = min(2, nchunks) if (interleave and parts == "all") else nchunks
                emit_all(proj_quanta(xc1, [wih1o_sb, wih1p_sb], srcs1, GT,
                                     range(nup)))
                projD_rest = proj_quanta(xc1, [wih1o_sb, wih1p_sb], srcs1, GT,
                                         range(nup, nchunks))

            if parts in ("all", "scans"):
                scan(xc1, whh1_sb, y1.ap(), extra=projD_rest)

    nc.compile()
    return nc


def _prep_inputs(x, lengths, weights, t_steps=T):
    active = (np.arange(T)[:, None] < np.asarray(lengths)[None, :]).astype(np.float32)
    ident = np.eye(128, dtype=f8)
    in_maps = []
    for c in range(NCORES):
        d, s = c // 4, c % 4
        bsl = slice(s * BS, (s + 1) * BS)
        pre = "f" if d == 0 else "b"
        xs = np.asarray(x[:, bsl, :], np.float32)
        am = active[:, bsl]
        if d == 1:
            xs = xs[::-1]
            am = am[::-1]
        xs = xs[:t_steps]
        am = am[:t_steps]

        W_ih0 = np.asarray(weights[f"{pre}W_ih0"], np.float32)
        W_hh0 = np.asarray(weights[f"{pre}W_hh0"], np.float32)
        W_ih1 = np.asarray(weights[f"{pre}W_ih1"], np.float32)
        W_hh1 = np.asarray(weights[f"{pre}W_hh1"], np.float32)
        b0 = np.asarray(weights[f"{pre}b0"], np.float32)
        b1 = np.asarray(weights[f"{pre}b1"], np.float32)
        own = W_ih1[:, :512] if d == 0 else W_ih1[:, 512:]
        par = W_ih1[:, 512:] if d == 0 else W_ih1[:, :512]

        amk = np.tile(am, (1, KH)).astype(bf16)          # [T, HB]
        mfull = np.zeros((t_steps + 1, 128, HB), bf16)
        mfull[:t_steps] = amk[:, None, :]

        in_maps.append({
            "xT": np.ascontiguousarray(xs.transpose(2, 0, 1)).astype(bf16),
            "maskd": mfull,
            "whh0T": np.ascontiguousarray(W_hh0.T.reshape(KH, 128, G) * WSCALE).astype(f8),
            "wih0T": np.ascontiguousarray(W_ih0.T.reshape(KH, 128, G) * WSCALE).astype(bf16),
            "whh1T": np.ascontiguousarray(W_hh1.T.reshape(KH, 128, G) * WSCALE).astype(f8),
            "wih1oT": np.ascontiguousarray(own.T.reshape(KH, 128, G) * WSCALE).astype(bf16),
            "wih1pT": np.ascontiguousarray(par.T.reshape(KH, 128, G) * WSCALE).astype(bf16),
            "identT": ident,
            "b0c": np.ascontiguousarray(b0.reshape(GT, 128) * WSCALE).astype(np.float32),
            "b1c": np.ascontiguousarray(b1.reshape(GT, 128) * WSCALE).astype(np.float32),
        })
    return in_maps


def _assemble(results, t_steps=T):
    out = np.zeros((t_steps, B, 2 * H), np.float32)
    for c in range(NCORES):
        d, s = c // 4, c % 4
        arr = results[c]["y1"].astype(np.float32).reshape(t_steps, 128, KH, BS)
        if d == 1:
            arr = arr[::-1]
        blk = arr.transpose(0, 3, 2, 1).reshape(t_steps, BS, H)
        out[:, s * BS:(s + 1) * BS, d * H:(d + 1) * H] = blk
    return out


def kernel(x, lengths, fW_ih0, fW_hh0, fb0, bW_ih0, bW_hh0, bb0,
           fW_ih1, fW_hh1, fb1, bW_ih1, bW_hh1, bb1, _t_steps=T,
           _want_trace=False, _dbg=False):
    weights = dict(fW_ih0=fW_ih0, fW_hh0=fW_hh0, fb0=fb0,
                   bW_ih0=bW_ih0, bW_hh0=bW_hh0, bb0=bb0,
                   fW_ih1=fW_ih1, fW_hh1=fW_hh1, fb1=fb1,
                   bW_ih1=bW_ih1, bW_hh1=bW_hh1, bb1=bb1)
    key = (_t_steps, _dbg)
    if key not in _compiled:
        _compiled[key] = _build(_t_steps, dbg=_dbg)
    nc = _compiled[key]
    in_maps = _prep_inputs(x, lengths, weights, _t_steps)
    res = bass_utils.run_bass_kernel_spmd(
        nc, in_maps, core_ids=list(range(NCORES)), trace=_want_trace)
    out = _assemble(res.results, _t_steps)
    if _want_trace or _dbg:
        kernel.last_results = res
    return out



# revision 31
# speedup vs baseline: 1.4812x; 1.4812x over previous
"""BiLSTM (2-layer, masked/ragged) Trainium2 kernel.

Sharding: 8 cores = 2 directions x 4 batch shards (16 each). Backward cores
receive time-reversed inputs from the host, so the device program is
direction-agnostic SPMD. Layer-0 outputs are exchanged between fwd/bwd
partner cores with pairwise AllGathers; each core computes layer-1 input
projections from its own half plus a time-reversed read of the partner half.

Scan step structure (per timestep, PSUM bank-parallel):
  - xw (precomputed input projection + bias) is injected into the four
    per-gate PSUM accumulators with identity matmuls (no DVE add).
  - 64 Whh matmuls (free dim 16) accumulate over it, ordered g,i,f,o so the
    sigmoid/tanh chain for c_t overlaps the tail of the PE stream.
  - Activations split per gate; h_{t+1}'s masked state is produced as
    (sigmoid(o)*mask) * tanh(c) so only one DVE op trails the last tanh.
Whh and the identity are fp8e4m3 (scaled x64, un-scaled via the free ACT
`scale` operand): FWL makes fp8 LDWEIGHTS 2x faster than bf16, and the
64-tile weight reload per step is the PE-side bound of the recurrence
(free-dim-16 matmuls are LDW-bound, not FLOP-bound). Wih/x/h stay bf16
(mixed fp8-stationary x bf16-moving matmuls are exact on TRN2); cell state
and the elementwise chain are fp32. Proj psum->SBUF activations are split
in halves so they can't head-of-line-block the scan's chain activations on
the ACT FIFO. DMAs batched 8 steps.
"""

import numpy as np
import ml_dtypes

import concourse.bass as bass
import concourse.bacc as bacc
import concourse.mybir as mybir
import concourse.tile as tile
from concourse import bass_utils

bf16 = ml_dtypes.bfloat16
f8 = ml_dtypes.float8_e4m3
f32 = mybir.dt.float32
bf = mybir.dt.bfloat16
fp8 = mybir.dt.float8e4
WSCALE = 64.0  # Whh stored as fp8e4m3 * 64; un-scaled in the gate activations

T, B, D, H = 512, 64, 512, 512
NCORES = 8
BS = B // 4          # 16 batch per core
G = 4 * H            # 2048 gates
GT = G // 128        # 16 gate tiles
KH = H // 128        # 4 k-chunks
HB = KH * BS         # 64 state cols
SC = 8               # scan steps per DMA batch
TC = 32              # proj timesteps per chunk
NCH = T // TC

SIG = mybir.ActivationFunctionType.Sigmoid
TANH = mybir.ActivationFunctionType.Tanh
IDENT = mybir.ActivationFunctionType.Identity

_compiled = {}


def _build(t_steps=T, dbg=False, nocc=False, parts="all", interleave=True, prefetch=True):
    nc = bacc.Bacc("TRN2", target_bir_lowering=False, debug=False,
                   num_devices=NCORES)
    nchunks = t_steps // TC

    xT = nc.dram_tensor("xT", (D, t_steps, BS), bf, kind="ExternalInput")
    maskd = nc.dram_tensor("maskd", (t_steps + 1, 128, HB), bf, kind="ExternalInput")
    whh0T = nc.dram_tensor("whh0T", (KH, 128, G), fp8, kind="ExternalInput")
    wih0T = nc.dram_tensor("wih0T", (KH, 128, G), bf, kind="ExternalInput")
    whh1T = nc.dram_tensor("whh1T", (KH, 128, G), fp8, kind="ExternalInput")
    wih1oT = nc.dram_tensor("wih1oT", (KH, 128, G), bf, kind="ExternalInput")
    wih1pT = nc.dram_tensor("wih1pT", (KH, 128, G), bf, kind="ExternalInput")
    identT = nc.dram_tensor("identT", (128, 128), fp8, kind="ExternalInput")
    b0c = nc.dram_tensor("b0c", (GT, 128), f32, kind="ExternalInput")
    b1c = nc.dram_tensor("b1c", (GT, 128), f32, kind="ExternalInput")
    y1 = nc.dram_tensor("y1", (t_steps, 128, HB), bf, kind="ExternalOutput")
    if dbg:
        y0o = nc.dram_tensor("y0o", (t_steps, 128, HB), bf, kind="ExternalOutput")
    with tile.TileContext(nc) as tc:
        with (
            tc.tile_pool(name="wpool", bufs=1) as wpool,
            tc.tile_pool(name="xpool", bufs=3) as xpool,
            tc.tile_pool(name="xcpool", bufs=4) as xcpool,
            tc.tile_pool(name="gpool", bufs=3) as gpool,
            tc.tile_pool(name="spool", bufs=6) as spool,
            tc.tile_pool(name="opool", bufs=2) as opool,
            tc.tile_pool(name="mpool", bufs=2) as mpool,
            tc.tile_pool(name="state", bufs=1) as state,
            tc.tile_pool(name="psA", bufs=3, space="PSUM") as psA,
            tc.tile_pool(name="psS", bufs=1, space="PSUM") as psS,
            tc.tile_pool(name="dram", bufs=1, space="DRAM") as dram,
        ):
            y0 = dram.tile([t_steps, 128, HB], bf)
            ag = dram.tile([2 * t_steps, 128, HB], bf)

            def load_w(name, src, dt=bf):
                t = wpool.tile([128, KH * G], dt, tag=name)
                for k in range(KH):
                    nc.sync.dma_start(t[:, k * G:(k + 1) * G], src.ap()[k])
                return t

            whh0_sb = load_w("whh0", whh0T, fp8)
            wih0_sb = load_w("wih0", wih0T)
            whh1_sb = load_w("whh1", whh1T, fp8)
            wih1o_sb = load_w("wih1o", wih1oT)
            wih1p_sb = load_w("wih1p", wih1pT)
            ident_sb = wpool.tile([128, 128], fp8, tag="ident")
            nc.sync.dma_start(ident_sb[:], identT.ap())
            bias_sb = wpool.tile([128, 2 * GT], f32, tag="bias")
            nc.sync.dma_start(bias_sb[:, 0:GT], b0c.ap().transpose([1, 0]))
            nc.sync.dma_start(bias_sb[:, GT:2 * GT], b1c.ap().transpose([1, 0]))

            # ---- input projections -> xwb dram ----
            # Emitted as a list of small "quanta" (thunks) so chunks beyond
            # the first two can be interleaved into the recurrent scan's PE
            # bubbles (the scan waits ~1us per step on the h-chain; proj
            # matmuls have no h dependency and fill that idle time).
            def proj_quanta(chmap, w_sbs, srcs, bias_col, chunks):
                nk = len(w_sbs) * KH
                quanta = []
                for ch in chunks:
                    t0 = ch * TC
                    state = {}

                    def dma_q(ch=ch, t0=t0, state=state):
                        rhs = xpool.tile([128, nk, TC, BS], bf, tag="projx",
                                         name="projx")
                        ji = 0
                        for w_sb, src in zip(w_sbs, srcs):
                            for k in range(KH):
                                nc.sync.dma_start(rhs[:, ji], src(k, t0))
                                ji += 1
                        state["rhs"] = rhs
                        # SBUF-resident destination chunk: Tile tracks SBUF
                        # deps reliably, making scan-interleaved production
                        # race-free (and skipping the xwb DRAM round trip).
                        state["xc"] = xcpool.tile([128, TC, GT * BS], bf,
                                                  tag="xchunk", name="xchunk")
                        chmap[ch] = state["xc"]
                    quanta.append(dma_q)

                    for g in range(GT):
                        halves = ([range(0, nk)] if nk <= 4 else
                                  [range(0, 4), range(4, nk)])
                        for hi, js in enumerate(halves):
                            def gate_q(t0=t0, g=g, js=js, hi=hi, nh=len(halves),
                                       state=state):
                                if hi == 0:
                                    state["ps"] = psA.tile(
                                        [128, TC * BS], f32, tag="psA",
                                        name="psA")
                                ps = state["ps"]
                                rhs = state["rhs"]
                                for ji in js:
                                    w_sb = w_sbs[ji // KH]
                                    k = ji % KH
                                    nc.tensor.matmul(
                                        ps[:],
                                        w_sb[:, k * G + g * 128: k * G + (g + 1) * 128],
                                        rhs[:, ji],
                                        start=(ji == 0),
                                        stop=(ji == nk - 1),
                                    )
                                if hi == nh - 1:
                                    # two half-ops: a 600ns ACT head-of-line-
                                    # blocks the scan chain's activations
                                    T2 = TC // 2
                                    for th in range(2):
                                        nc.scalar.activation(
                                            state["xc"][:, th * T2:(th + 1) * T2,
                                                        g * BS:(g + 1) * BS],
                                            ps[:, th * T2 * BS:(th + 1) * T2 * BS]
                                            .rearrange("p (t b) -> p t b", t=T2),
                                            IDENT,
                                            bias=bias_sb[:, bias_col + g:
                                                         bias_col + g + 1],
                                        )
                            quanta.append(gate_q)
                return quanta

            def emit_all(quanta):
                for q in quanta:
                    q()

            srcs0 = [lambda k, t0: xT.ap()[k * 128:(k + 1) * 128, t0:t0 + TC, :]]
            xc0, xc1 = {}, {}
            projA_rest = []
            if parts in ("all", "proj"):
                nup = min(2, nchunks) if (interleave and parts == "all") else nchunks
                emit_all(proj_quanta(xc0, [wih0_sb], srcs0, 0, range(nup)))
                projA_rest = proj_quanta(xc0, [wih0_sb], srcs0, 0,
                                         range(nup, nchunks))

            # ---- recurrent scan ----
            # natural gate-type order in xw columns / weight columns: i,f,g,o
            QI = {"i": 0, "f": 1, "g": 2, "o": 3}

            def scan(chmap, whh_sb, y_dst, extra=()):
                h0 = state.tile([128, HB], bf, tag="h0")
                cst = state.tile([128, HB], f32, tag="cst")
                nc.gpsimd.memset(h0[:], 0.0)
                nc.gpsimd.memset(cst[:], 0.0)
                hprev = h0[:]
                extra = list(extra)
                nex = len(extra)
                spread = max(1, t_steps - 2 * TC)

                def load_win(t0):
                    mk = mpool.tile([128, SC + 1, HB], bf, tag="mk", name="mk")
                    nc.sync.dma_start(
                        mk[:], maskd.ap()[t0:t0 + SC + 1].transpose([1, 0, 2]))
                    return mk

                nxt = load_win(0) if prefetch else None
                mk = ob = None
                for t in range(t_steps):
                    j = t % SC
                    if j == 0:
                        if prefetch:
                            mk = nxt
                            if t + SC < t_steps:
                                nxt = load_win(t + SC)
                        else:
                            mk = load_win(t)
                        ob = opool.tile([128, SC, HB], bf, tag="ob", name="ob")
                    xw = chmap[t // TC]
                    tr = t % TC
                    # c state mask (c *= m[t]); h mask was folded into hprev
                    if t > 0:
                        nc.vector.tensor_mul(cst[:], cst[:], mk[:, j])
                    ps = {q: psS.tile([128, 512], f32, tag=f"ps{q}",
                                      name=f"ps{q}") for q in "ifgo"}
                    for q in "ifgo":
                        nc.tensor.matmul(
                            ps[q][:, 0:HB], ident_sb[:],
                            xw[:, tr, QI[q] * HB:(QI[q] + 1) * HB],
                            start=True, stop=False, skip_group_check=True)
                    for q in "gifo":
                        qi = QI[q]
                        for gt in range(4):
                            for k in range(KH):
                                nc.tensor.matmul(
                                    ps[q][:, gt * BS:(gt + 1) * BS],
                                    whh_sb[:, k * G + qi * 512 + gt * 128:
                                           k * G + qi * 512 + (gt + 1) * 128],
                                    hprev[:, k * BS:(k + 1) * BS],
                                    start=False, stop=(k == KH - 1),
                                    skip_group_check=True)
                    inv = 1.0 / WSCALE
                    tg = spool.tile([128, HB], f32, tag="tg")
                    nc.scalar.activation(tg[:], ps["g"][:, 0:HB], TANH, scale=inv)
                    si = spool.tile([128, HB], f32, tag="si")
                    nc.scalar.activation(si[:], ps["i"][:, 0:HB], SIG, scale=inv)
                    ig = spool.tile([128, HB], f32, tag="ig")
                    nc.vector.tensor_mul(ig[:], si[:], tg[:])
                    sf = spool.tile([128, HB], f32, tag="sf")
                    nc.scalar.activation(sf[:], ps["f"][:, 0:HB], SIG, scale=inv)
                    fc = spool.tile([128, HB], f32, tag="fc")
                    nc.vector.tensor_mul(fc[:], sf[:], cst[:])
                    nc.vector.tensor_add(cst[:], fc[:], ig[:])
                    so = spool.tile([128, HB], f32, tag="so")
                    nc.scalar.activation(so[:], ps["o"][:, 0:HB], SIG, scale=inv)
                    tc2 = spool.tile([128, HB], f32, tag="tc2")
                    nc.scalar.activation(tc2[:], cst[:], TANH)
                    if t + 1 < t_steps:
                        # next-state path first: hm = (so*m[t+1]) * tanh(c)
                        som = spool.tile([128, HB], f32, tag="som")
                        nc.vector.tensor_mul(som[:], so[:], mk[:, j + 1])
                        hm = spool.tile([128, HB], bf, tag="hm")
                        nc.vector.tensor_mul(hm[:], som[:], tc2[:])
                        hprev = hm[:]
                    h2 = spool.tile([128, HB], f32, tag="h2")
                    nc.vector.tensor_mul(h2[:], so[:], tc2[:])
                    nc.vector.tensor_mul(ob[:, j], h2[:], mk[:, j])
                    if j == SC - 1:
                        nc.sync.dma_start(
                            y_dst[t - SC + 1:t + 1].transpose([1, 0, 2]), ob[:])
                    # fill this step's PE bubble with interleaved proj work
                    q0 = nex * t // spread
                    q1 = nex * (t + 1) // spread
                    for q in extra[q0:min(q1, nex)]:
                        q()

            if parts in ("all", "scans"):
                scan(xc0, whh0_sb, y0, extra=projA_rest)
            if dbg:
                nc.sync.dma_start(y0o.ap()[:], y0[:])

            # ---- exchange (pairwise fwd<->bwd) ----
            # Quarter-split AllGather: quarter q's input rows are final at
            # L0-scan step (q+1)*QT, so quarters 0-2 transfer while the scan
            # is still running; only the last quarter (~70us) stays exposed
            # between the scans. (One full-tensor AllGather was ~260us of
            # serial critical path.)
            projD_rest = []
            if parts in ("all", "proj"):
                QN = 4 if t_steps % (4 * TC) == 0 else 1
                QT = t_steps // QN
                agq = [ag[2 * q * QT:2 * (q + 1) * QT] for q in range(QN)]
                if nocc:
                    for q in range(QN):
                        nc.sync.dma_start(agq[q][0:QT],
                                          y0[q * QT:(q + 1) * QT])
                        nc.sync.dma_start(agq[q][QT:2 * QT],
                                          y0[q * QT:(q + 1) * QT])
                    partner_q = nc.snap(QT)
                else:
                    for q in range(QN):
                        nc.gpsimd.collective_compute(
                            "AllGather", mybir.AluOpType.bypass,
                            ins=[y0[q * QT:(q + 1) * QT].opt()],
                            outs=[agq[q].opt()],
                            replica_groups=[[0, 4], [1, 5], [2, 6], [3, 7]],
                        )
                    partner_q = nc.snap(
                        ((nc.partition_id() // 4 + 1) % 2) * QT)

                def par_src(k, t0):
                    # partner rows, time-reversed: own step tau needs partner
                    # row (T-1-tau); rows [T-TC-t0, T-t0) reversed. The TC
                    # range never straddles a quarter (TC | QT).
                    gs = t_steps - TC - t0
                    q, loc = gs // QT, gs % QT
                    return (agq[q][bass.ds(partner_q + loc, TC)]
                            [::-1, :, k * BS:(k + 1) * BS].transpose([1, 0, 2]))

                srcs1 = [
                    lambda k, t0: y0[t0:t0 + TC, :, k * BS:(k + 1) * BS].transpose([1, 0, 2]),
                    par_src,
                ]
                nup = min(2, nchunks) if (interleave and parts == "all") else nchunks
                emit_all(proj_quanta(xc1, [wih1o_sb, wih1p_sb], srcs1, GT,
                                     range(nup)))
                projD_rest = proj_quanta(xc1, [wih1o_sb, wih1p_sb], srcs1, GT,
                                         range(nup, nchunks))

            if parts in ("all", "scans"):
                scan(xc1, whh1_sb, y1.ap(), extra=projD_rest)

    nc.compile()
    return nc


def _prep_inputs(x, lengths, weights, t_steps=T):
    active = (np.arange(T)[:, None] < np.asarray(lengths)[None, :]).astype(np.float32)
    ident = np.eye(128, dtype=f8)
    in_maps = []
    for c in range(NCORES):
        d, s = c // 4, c % 4
        bsl = slice(s * BS, (s + 1) * BS)
        pre = "f" if d == 0 else "b"
        xs = np.asarray(x[:, bsl, :], np.float32)
        am = active[:, bsl]
        if d == 1:
            xs = xs[::-1]
            am = am[::-1]
        xs = xs[:t_steps]
        am = am[:t_steps]

        W_ih0 = np.asarray(weights[f"{pre}W_ih0"], np.float32)
        W_hh0 = np.asarray(weights[f"{pre}W_hh0"], np.float32)
        W_ih1 = np.asarray(weights[f"{pre}W_ih1"], np.float32)
        W_hh1 = np.asarray(weights[f"{pre}W_hh1"], np.float32)
        b0 = np.asarray(weights[f"{pre}b0"], np.float32)
        b1 = np.asarray(weights[f"{pre}b1"], np.float32)
        own = W_ih1[:, :512] if d == 0 else W_ih1[:, 512:]
        par = W_ih1[:, 512:] if d == 0 else W_ih1[:, :512]

        amk = np.tile(am, (1, KH)).astype(bf16)          # [T, HB]
        mfull = np.zeros((t_steps + 1, 128, HB), bf16)
        mfull[:t_steps] = amk[:, None, :]

        in_maps.append({
            "xT": np.ascontiguousarray(xs.transpose(2, 0, 1)).astype(bf16),
            "maskd": mfull,
            "whh0T": np.ascontiguousarray(W_hh0.T.reshape(KH, 128, G) * WSCALE).astype(f8),
            "wih0T": np.ascontiguousarray(W_ih0.T.reshape(KH, 128, G) * WSCALE).astype(bf16),
            "whh1T": np.ascontiguousarray(W_hh1.T.reshape(KH, 128, G) * WSCALE).astype(f8),
            "wih1oT": np.ascontiguousarray(own.T.reshape(KH, 128, G) * WSCALE).astype(bf16),
            "wih1pT": np.ascontiguousarray(par.T.reshape(KH, 128, G) * WSCALE).astype(bf16),
            "identT": ident,
            "b0c": np.ascontiguousarray(b0.reshape(GT, 128) * WSCALE).astype(np.float32),
            "b1c": np.ascontiguousarray(b1.reshape(GT, 128) * WSCALE).astype(np.float32),
        })
    return in_maps


def _assemble(results, t_steps=T):
    out = np.zeros((t_steps, B, 2 * H), np.float32)
    for c in range(NCORES):
        d, s = c // 4, c % 4
        arr = results[c]["y1"].astype(np.float32).reshape(t_steps, 128, KH, BS)
        if d == 1:
            arr = arr[::-1]
        blk = arr.transpose(0, 3, 2, 1).reshape(t_steps, BS, H)
        out[:, s * BS:(s + 1) * BS, d * H:(d + 1) * H] = blk
    return out


def kernel(x, lengths, fW_ih0, fW_hh0, fb0, bW_ih0, bW_hh0, bb0,
           fW_ih1, fW_hh1, fb1, bW_ih1, bW_hh1, bb1, _t_steps=T,
           _want_trace=False, _dbg=False):
    weights = dict(fW_ih0=fW_ih0, fW_hh0=fW_hh0, fb0=fb0,
                   bW_ih0=bW_ih0, bW_hh0=bW_hh0, bb0=bb0,
                   fW_ih1=fW_ih1, fW_hh1=fW_hh1, fb1=fb1,
                   bW_ih1=bW_ih1, bW_hh1=bW_hh1, bb1=bb1)
    key = (_t_steps, _dbg)
    if key not in _compiled:
        _compiled[key] = _build(_t_steps, dbg=_dbg)
    nc = _compiled[key]
    in_maps = _prep_inputs(x, lengths, weights, _t_steps)
    res = bass_utils.run_bass_kernel_spmd(
        nc, in_maps, core_ids=list(range(NCORES)), trace=_want_trace)
    out = _assemble(res.results, _t_steps)
    if _want_trace or _dbg:
        kernel.last_results = res
    return out



# revision 33
# speedup vs baseline: 1.7192x; 1.1607x over previous
"""BiLSTM (2-layer, masked/ragged) Trainium2 kernel.

Sharding: 8 cores = 2 directions x 4 batch shards (16 each). Backward cores
receive time-reversed inputs from the host, so the device program is
direction-agnostic SPMD. Layer-0 outputs are exchanged between fwd/bwd
partner cores with pairwise AllGathers, split into four quarter-tensor
collectives so quarters 0-2 transfer while the layer-0 scan is still
running (only the last ~70us stays exposed between the scans); each core
computes layer-1 input projections from its own half plus a time-reversed
read of the partner half.

Scan step structure (per timestep, PSUM bank-parallel):
  - xw (precomputed input projection + bias) is injected into the four
    per-gate PSUM accumulators with identity matmuls (no DVE add).
  - 64 Whh matmuls (free dim 16) accumulate over it, ordered g,i,f,o so the
    sigmoid/tanh chain for c_t overlaps the tail of the PE stream.
  - Activations split per gate; h_{t+1}'s masked state is produced as
    (sigmoid(o)*mask) * tanh(c) so only one DVE op trails the last tanh.
Whh and the identity are fp8e4m3 (scaled x64, un-scaled via the free ACT
`scale` operand): FWL makes fp8 LDWEIGHTS 2x faster than bf16, and the
64-tile weight reload per step is the PE-side bound of the recurrence
(free-dim-16 matmuls are LDW-bound, not FLOP-bound). Wih/x/h stay bf16
(mixed fp8-stationary x bf16-moving matmuls are exact on TRN2); cell state
and the elementwise chain are fp32. Proj psum->SBUF activations are split
in halves so they can't head-of-line-block the scan's chain activations on
the ACT FIFO. DMAs batched 8 steps.
"""

import numpy as np
import ml_dtypes

import concourse.bass as bass
import concourse.bacc as bacc
import concourse.mybir as mybir
import concourse.tile as tile
from concourse import bass_utils

bf16 = ml_dtypes.bfloat16
f8 = ml_dtypes.float8_e4m3
f32 = mybir.dt.float32
bf = mybir.dt.bfloat16
fp8 = mybir.dt.float8e4
WSCALE = 64.0  # Whh stored as fp8e4m3 * 64; un-scaled in the gate activations

T, B, D, H = 512, 64, 512, 512
NCORES = 8
BS = B // 4          # 16 batch per core
G = 4 * H            # 2048 gates
GT = G // 128        # 16 gate tiles
KH = H // 128        # 4 k-chunks
HB = KH * BS         # 64 state cols
SC = 8               # scan steps per DMA batch
TC = 32              # proj timesteps per chunk
NCH = T // TC

SIG = mybir.ActivationFunctionType.Sigmoid
TANH = mybir.ActivationFunctionType.Tanh
IDENT = mybir.ActivationFunctionType.Identity

_compiled = {}


def _build(t_steps=T, dbg=False, nocc=False, parts="all", interleave=True, prefetch=True):
    nc = bacc.Bacc("TRN2", target_bir_lowering=False, debug=False,
                   num_devices=NCORES)
    nchunks = t_steps // TC

    xT = nc.dram_tensor("xT", (D, t_steps, BS), bf, kind="ExternalInput")
    maskd = nc.dram_tensor("maskd", (t_steps + 1, 128, HB), bf, kind="ExternalInput")
    whh0T = nc.dram_tensor("whh0T", (KH, 128, G), fp8, kind="ExternalInput")
    wih0T = nc.dram_tensor("wih0T", (KH, 128, G), bf, kind="ExternalInput")
    whh1T = nc.dram_tensor("whh1T", (KH, 128, G), fp8, kind="ExternalInput")
    wih1oT = nc.dram_tensor("wih1oT", (KH, 128, G), bf, kind="ExternalInput")
    wih1pT = nc.dram_tensor("wih1pT", (KH, 128, G), bf, kind="ExternalInput")
    identT = nc.dram_tensor("identT", (128, 128), fp8, kind="ExternalInput")
    b0c = nc.dram_tensor("b0c", (GT, 128), f32, kind="ExternalInput")
    b1c = nc.dram_tensor("b1c", (GT, 128), f32, kind="ExternalInput")
    y1 = nc.dram_tensor("y1", (t_steps, 128, HB), bf, kind="ExternalOutput")
    if dbg:
        y0o = nc.dram_tensor("y0o", (t_steps, 128, HB), bf, kind="ExternalOutput")
    with tile.TileContext(nc) as tc:
        with (
            tc.tile_pool(name="wpool", bufs=1) as wpool,
            tc.tile_pool(name="xpool", bufs=3) as xpool,
            tc.tile_pool(name="xcpool", bufs=4) as xcpool,
            tc.tile_pool(name="gpool", bufs=3) as gpool,
            tc.tile_pool(name="spool", bufs=6) as spool,
            tc.tile_pool(name="opool", bufs=2) as opool,
            tc.tile_pool(name="mpool", bufs=2) as mpool,
            tc.tile_pool(name="state", bufs=1) as state,
            tc.tile_pool(name="psA", bufs=3, space="PSUM") as psA,
            tc.tile_pool(name="psS", bufs=1, space="PSUM") as psS,
            tc.tile_pool(name="dram", bufs=1, space="DRAM") as dram,
        ):
            y0 = dram.tile([t_steps, 128, HB], bf)
            ag = dram.tile([2 * t_steps, 128, HB], bf)

            def load_w(name, src, dt=bf):
                t = wpool.tile([128, KH * G], dt, tag=name)
                for k in range(KH):
                    nc.sync.dma_start(t[:, k * G:(k + 1) * G], src.ap()[k])
                return t

            whh0_sb = load_w("whh0", whh0T, fp8)
            wih0_sb = load_w("wih0", wih0T)
            whh1_sb = load_w("whh1", whh1T, fp8)
            wih1o_sb = load_w("wih1o", wih1oT)
            wih1p_sb = load_w("wih1p", wih1pT)
            ident_sb = wpool.tile([128, 128], fp8, tag="ident")
            nc.sync.dma_start(ident_sb[:], identT.ap())
            bias_sb = wpool.tile([128, 2 * GT], f32, tag="bias")
            nc.sync.dma_start(bias_sb[:, 0:GT], b0c.ap().transpose([1, 0]))
            nc.sync.dma_start(bias_sb[:, GT:2 * GT], b1c.ap().transpose([1, 0]))

            # ---- input projections -> xwb dram ----
            # Emitted as a list of small "quanta" (thunks) so chunks beyond
            # the first two can be interleaved into the recurrent scan's PE
            # bubbles (the scan waits ~1us per step on the h-chain; proj
            # matmuls have no h dependency and fill that idle time).
            def proj_quanta(chmap, w_sbs, srcs, bias_col, chunks):
                nk = len(w_sbs) * KH
                quanta = []
                for ch in chunks:
                    t0 = ch * TC
                    state = {}

                    def dma_q(ch=ch, t0=t0, state=state):
                        rhs = xpool.tile([128, nk, TC, BS], bf, tag="projx",
                                         name="projx")
                        ji = 0
                        for w_sb, src in zip(w_sbs, srcs):
                            for k in range(KH):
                                nc.sync.dma_start(rhs[:, ji], src(k, t0))
                                ji += 1
                        state["rhs"] = rhs
                        # SBUF-resident destination chunk: Tile tracks SBUF
                        # deps reliably, making scan-interleaved production
                        # race-free (and skipping the xwb DRAM round trip).
                        state["xc"] = xcpool.tile([128, TC, GT * BS], bf,
                                                  tag="xchunk", name="xchunk")
                        chmap[ch] = state["xc"]
                    quanta.append(dma_q)

                    for g in range(GT):
                        halves = ([range(0, nk)] if nk <= 4 else
                                  [range(0, 4), range(4, nk)])
                        for hi, js in enumerate(halves):
                            def gate_q(t0=t0, g=g, js=js, hi=hi, nh=len(halves),
                                       state=state):
                                if hi == 0:
                                    state["ps"] = psA.tile(
                                        [128, TC * BS], f32, tag="psA",
                                        name="psA")
                                ps = state["ps"]
                                rhs = state["rhs"]
                                for ji in js:
                                    w_sb = w_sbs[ji // KH]
                                    k = ji % KH
                                    nc.tensor.matmul(
                                        ps[:],
                                        w_sb[:, k * G + g * 128: k * G + (g + 1) * 128],
                                        rhs[:, ji],
                                        start=(ji == 0),
                                        stop=(ji == nk - 1),
                                    )
                                if hi == nh - 1:
                                    # two half-ops: a 600ns ACT head-of-line-
                                    # blocks the scan chain's activations
                                    T2 = TC // 2
                                    for th in range(2):
                                        nc.scalar.activation(
                                            state["xc"][:, th * T2:(th + 1) * T2,
                                                        g * BS:(g + 1) * BS],
                                            ps[:, th * T2 * BS:(th + 1) * T2 * BS]
                                            .rearrange("p (t b) -> p t b", t=T2),
                                            IDENT,
                                            bias=bias_sb[:, bias_col + g:
                                                         bias_col + g + 1],
                                        )
                            quanta.append(gate_q)
                return quanta

            def emit_all(quanta):
                for q in quanta:
                    q()

            srcs0 = [lambda k, t0: xT.ap()[k * 128:(k + 1) * 128, t0:t0 + TC, :]]
            xc0, xc1 = {}, {}
            projA_rest = []
            if parts in ("all", "proj"):
                nup = min(1, nchunks) if (interleave and parts == "all") else nchunks
                emit_all(proj_quanta(xc0, [wih0_sb], srcs0, 0, range(nup)))
                projA_rest = proj_quanta(xc0, [wih0_sb], srcs0, 0,
                                         range(nup, nchunks))

            # ---- recurrent scan ----
            # natural gate-type order in xw columns / weight columns: i,f,g,o
            QI = {"i": 0, "f": 1, "g": 2, "o": 3}

            def scan(chmap, whh_sb, y_dst, extra=()):
                h0 = state.tile([128, HB], bf, tag="h0")
                cst = state.tile([128, HB], f32, tag="cst")
                nc.gpsimd.memset(h0[:], 0.0)
                nc.gpsimd.memset(cst[:], 0.0)
                hprev = h0[:]
                extra = list(extra)
                nex = len(extra)
                spread = max(1, t_steps - 2 * TC)

                def load_win(t0):
                    mk = mpool.tile([128, SC + 1, HB], bf, tag="mk", name="mk")
                    nc.sync.dma_start(
                        mk[:], maskd.ap()[t0:t0 + SC + 1].transpose([1, 0, 2]))
                    return mk

                nxt = load_win(0) if prefetch else None
                mk = ob = None
                for t in range(t_steps):
                    j = t % SC
                    if j == 0:
                        if prefetch:
                            mk = nxt
                            if t + SC < t_steps:
                                nxt = load_win(t + SC)
                        else:
                            mk = load_win(t)
                        ob = opool.tile([128, SC, HB], bf, tag="ob", name="ob")
                    xw = chmap[t // TC]
                    tr = t % TC
                    # c state mask (c *= m[t]); h mask was folded into hprev
                    if t > 0:
                        nc.vector.tensor_mul(cst[:], cst[:], mk[:, j])
                    ps = {q: psS.tile([128, 512], f32, tag=f"ps{q}",
                                      name=f"ps{q}") for q in "ifgo"}
                    for q in "ifgo":
                        nc.tensor.matmul(
                            ps[q][:, 0:HB], ident_sb[:],
                            xw[:, tr, QI[q] * HB:(QI[q] + 1) * HB],
                            start=True, stop=False, skip_group_check=True)
                    for q in "gifo":
                        qi = QI[q]
                        for gt in range(4):
                            for k in range(KH):
                                nc.tensor.matmul(
                                    ps[q][:, gt * BS:(gt + 1) * BS],
                                    whh_sb[:, k * G + qi * 512 + gt * 128:
                                           k * G + qi * 512 + (gt + 1) * 128],
                                    hprev[:, k * BS:(k + 1) * BS],
                                    start=False, stop=(k == KH - 1),
                                    skip_group_check=True)
                    inv = 1.0 / WSCALE
                    tg = spool.tile([128, HB], f32, tag="tg")
                    nc.scalar.activation(tg[:], ps["g"][:, 0:HB], TANH, scale=inv)
                    si = spool.tile([128, HB], f32, tag="si")
                    nc.scalar.activation(si[:], ps["i"][:, 0:HB], SIG, scale=inv)
                    ig = spool.tile([128, HB], f32, tag="ig")
                    nc.vector.tensor_mul(ig[:], si[:], tg[:])
                    sf = spool.tile([128, HB], f32, tag="sf")
                    nc.scalar.activation(sf[:], ps["f"][:, 0:HB], SIG, scale=inv)
                    fc = spool.tile([128, HB], f32, tag="fc")
                    nc.vector.tensor_mul(fc[:], sf[:], cst[:])
                    nc.vector.tensor_add(cst[:], fc[:], ig[:])
                    so = spool.tile([128, HB], f32, tag="so")
                    nc.scalar.activation(so[:], ps["o"][:, 0:HB], SIG, scale=inv)
                    tc2 = spool.tile([128, HB], f32, tag="tc2")
                    nc.scalar.activation(tc2[:], cst[:], TANH)
                    if t + 1 < t_steps:
                        # next-state path first: hm = (so*m[t+1]) * tanh(c)
                        som = spool.tile([128, HB], f32, tag="som")
                        nc.vector.tensor_mul(som[:], so[:], mk[:, j + 1])
                        hm = spool.tile([128, HB], bf, tag="hm")
                        nc.vector.tensor_mul(hm[:], som[:], tc2[:])
                        hprev = hm[:]
                    h2 = spool.tile([128, HB], f32, tag="h2")
                    nc.vector.tensor_mul(h2[:], so[:], tc2[:])
                    nc.vector.tensor_mul(ob[:, j], h2[:], mk[:, j])
                    if j == SC - 1:
                        nc.sync.dma_start(
                            y_dst[t - SC + 1:t + 1].transpose([1, 0, 2]), ob[:])
                    # fill this step's PE bubble with interleaved proj work
                    q0 = nex * t // spread
                    q1 = nex * (t + 1) // spread
                    for q in extra[q0:min(q1, nex)]:
                        q()

            if parts in ("all", "scans"):
                scan(xc0, whh0_sb, y0, extra=projA_rest)
            if dbg:
                nc.sync.dma_start(y0o.ap()[:], y0[:])

            # ---- exchange (pairwise fwd<->bwd) ----
            # Quarter-split AllGather: quarter q's input rows are final at
            # L0-scan step (q+1)*QT, so quarters 0-2 transfer while the scan
            # is still running; only the last quarter (~70us) stays exposed
            # between the scans. (One full-tensor AllGather was ~260us of
            # serial critical path.)
            projD_rest = []
            if parts in ("all", "proj"):
                QN = 4 if t_steps % (4 * TC) == 0 else 1
                QT = t_steps // QN
                agq = [ag[2 * q * QT:2 * (q + 1) * QT] for q in range(QN)]
                if nocc:
                    for q in range(QN):
                        nc.sync.dma_start(agq[q][0:QT],
                                          y0[q * QT:(q + 1) * QT])
                        nc.sync.dma_start(agq[q][QT:2 * QT],
                                          y0[q * QT:(q + 1) * QT])
                    partner_q = nc.snap(QT)
                else:
                    for q in range(QN):
                        nc.gpsimd.collective_compute(
                            "AllGather", mybir.AluOpType.bypass,
                            ins=[y0[q * QT:(q + 1) * QT].opt()],
                            outs=[agq[q].opt()],
                            replica_groups=[[0, 4], [1, 5], [2, 6], [3, 7]],
                        )
                    partner_q = nc.snap(
                        ((nc.partition_id() // 4 + 1) % 2) * QT)

                def par_src(k, t0):
                    # partner rows, time-reversed: own step tau needs partner
                    # row (T-1-tau); rows [T-TC-t0, T-t0) reversed. The TC
                    # range never straddles a quarter (TC | QT).
                    gs = t_steps - TC - t0
                    q, loc = gs // QT, gs % QT
                    return (agq[q][bass.ds(partner_q + loc, TC)]
                            [::-1, :, k * BS:(k + 1) * BS].transpose([1, 0, 2]))

                srcs1 = [
                    lambda k, t0: y0[t0:t0 + TC, :, k * BS:(k + 1) * BS].transpose([1, 0, 2]),
                    par_src,
                ]
                nup = min(1, nchunks) if (interleave and parts == "all") else nchunks
                emit_all(proj_quanta(xc1, [wih1o_sb, wih1p_sb], srcs1, GT,
                                     range(nup)))
                projD_rest = proj_quanta(xc1, [wih1o_sb, wih1p_sb], srcs1, GT,
                                         range(nup, nchunks))

            if parts in ("all", "scans"):
                scan(xc1, whh1_sb, y1.ap(), extra=projD_rest)

    nc.compile()
    return nc


def _prep_inputs(x, lengths, weights, t_steps=T):
    active = (np.arange(T)[:, None] < np.asarray(lengths)[None, :]).astype(np.float32)
    ident = np.eye(128, dtype=f8)
    in_maps = []
    for c in range(NCORES):
        d, s = c // 4, c % 4
        bsl = slice(s * BS, (s + 1) * BS)
        pre = "f" if d == 0 else "b"
        xs = np.asarray(x[:, bsl, :], np.float32)
        am = active[:, bsl]
        if d == 1:
            xs = xs[::-1]
            am = am[::-1]
        xs = xs[:t_steps]
        am = am[:t_steps]

        W_ih0 = np.asarray(weights[f"{pre}W_ih0"], np.float32)
        W_hh0 = np.asarray(weights[f"{pre}W_hh0"], np.float32)
        W_ih1 = np.asarray(weights[f"{pre}W_ih1"], np.float32)
        W_hh1 = np.asarray(weights[f"{pre}W_hh1"], np.float32)
        b0 = np.asarray(weights[f"{pre}b0"], np.float32)
        b1 = np.asarray(weights[f"{pre}b1"], np.float32)
        own = W_ih1[:, :512] if d == 0 else W_ih1[:, 512:]
        par = W_ih1[:, 512:] if d == 0 else W_ih1[:, :512]

        amk = np.tile(am, (1, KH)).astype(bf16)          # [T, HB]
        mfull = np.zeros((t_steps + 1, 128, HB), bf16)
        mfull[:t_steps] = amk[:, None, :]

        in_maps.append({
            "xT": np.ascontiguousarray(xs.transpose(2, 0, 1)).astype(bf16),
            "maskd": mfull,
            "whh0T": np.ascontiguousarray(W_hh0.T.reshape(KH, 128, G) * WSCALE).astype(f8),
            "wih0T": np.ascontiguousarray(W_ih0.T.reshape(KH, 128, G) * WSCALE).astype(bf16),
            "whh1T": np.ascontiguousarray(W_hh1.T.reshape(KH, 128, G) * WSCALE).astype(f8),
            "wih1oT": np.ascontiguousarray(own.T.reshape(KH, 128, G) * WSCALE).astype(bf16),
            "wih1pT": np.ascontiguousarray(par.T.reshape(KH, 128, G) * WSCALE).astype(bf16),
            "identT": ident,
            "b0c": np.ascontiguousarray(b0.reshape(GT, 128) * WSCALE).astype(np.float32),
            "b1c": np.ascontiguousarray(b1.reshape(GT, 128) * WSCALE).astype(np.float32),
        })
    return in_maps


def _assemble(results, t_steps=T):
    out = np.zeros((t_steps, B, 2 * H), np.float32)
    for c in range(NCORES):
        d, s = c // 4, c % 4
        arr = results[c]["y1"].astype(np.float32).reshape(t_steps, 128, KH, BS)
        if d == 1:
            arr = arr[::-1]
        blk = arr.transpose(0, 3, 2, 1).reshape(t_steps, BS, H)
        out[:, s * BS:(s + 1) * BS, d * H:(d + 1) * H] = blk
    return out


def kernel(x, lengths, fW_ih0, fW_hh0, fb0, bW_ih0, bW_hh0, bb0,
           fW_ih1, fW_hh1, fb1, bW_ih1, bW_hh1, bb1, _t_steps=T,
           _want_trace=False, _dbg=False):
    weights = dict(fW_ih0=fW_ih0, fW_hh0=fW_hh0, fb0=fb0,
                   bW_ih0=bW_ih0, bW_hh0=bW_hh0, bb0=bb0,
                   fW_ih1=fW_ih1, fW_hh1=fW_hh1, fb1=fb1,
                   bW_ih1=bW_ih1, bW_hh1=bW_hh1, bb1=bb1)
    key = (_t_steps, _dbg)
    if key not in _compiled:
        _compiled[key] = _build(_t_steps, dbg=_dbg)
    nc = _compiled[key]
    in_maps = _prep_inputs(x, lengths, weights, _t_steps)
    res = bass_utils.run_bass_kernel_spmd(
        nc, in_maps, core_ids=list(range(NCORES)), trace=_want_trace)
    out = _assemble(res.results, _t_steps)
    if _want_trace or _dbg:
        kernel.last_results = res
    return out

